# revision 3
# baseline (speedup 1.0000x reference)
"""MoE (dense-act-dense, top-4 of 8 experts) Trainium2 kernel.

Strategy (expert-parallel, host-side dispatch):
  - The forward combine weight is exactly 1.0 (straight-through gate trick in
    the reference), so out[n] = sum_{e in top4(n)} expert_e(x[n]).
  - Host computes the tiny gate matmul + top-4 routing (0.05% of FLOPs) and
    dispatches tokens: core e receives the tokens routed to expert e
    (capacity-padded), plus expert e's weights. This is the sharding step.
  - Each of the 8 cores runs a dense 2-layer MLP (relu between) on its tokens:
      h = relu(w1[e] @ x) ; y = w2[e] @ h
    as two chained fp32r GEMMs (fp32 data, FP22 multiply, fp32 accumulate).
  - Host scatter-adds per-expert outputs back (weight 1.0 per selection).

Per-core device layouts (everything pre-transposed on host for contiguous DMA):
  xT  [D, C] f32r : routed tokens, transposed
  w1t [D, H] f32r : w1[e].T
  w2t [H, O] f32r : w2[e].T
  yT  [O, C] f32  : expert output, transposed
"""

import numpy as np
from contextlib import ExitStack

import concourse.bass as bass
import concourse.tile as tile
from concourse import bacc, mybir
from concourse import bass_utils

F32 = mybir.dt.float32
F32R = mybir.dt.float32r
P = 128

TOP_K = 4
D, H, O, E = 2048, 1024, 2048, 8
NT = 384          # token tile (matmul moving free dim); >=256 keeps fp32r at 1 cyc/row
NSUB = 1          # n-subtiles sharing one stationary weight load
X_BUFS = 2
H_BUFS = 2
Y_BUFS = 4

_NC_CACHE = {}


def build_expert_kernel(C):
    """Per-core program: dense [C, D] @ [D, H] -> relu -> @ [H, O] in fp32r."""
    DC, HC, OC = D // P, H // P, O // P
    NTILES = C // (NT * NSUB)
    nc = bacc.Bacc("TRN2", target_bir_lowering=False, debug=False, num_devices=E)
    xT = nc.dram_tensor("xT", [D, C], F32R, kind="ExternalInput").ap()
    w1t = nc.dram_tensor("w1t", [D, H], F32R, kind="ExternalInput").ap()
    w2t = nc.dram_tensor("w2t", [H, O], F32R, kind="ExternalInput").ap()
    yT = nc.dram_tensor("yT", [O, C], F32, kind="ExternalOutput").ap()

    with tile.TileContext(nc) as tc, ExitStack() as ctx:
        wpool = ctx.enter_context(tc.tile_pool(name="w", bufs=1))
        xpool = ctx.enter_context(tc.tile_pool(name="x", bufs=X_BUFS))
        hpool = ctx.enter_context(tc.tile_pool(name="h", bufs=H_BUFS))
        ypool = ctx.enter_context(tc.tile_pool(name="y", bufs=Y_BUFS))
        ps1 = ctx.enter_context(tc.tile_pool(name="ps1", bufs=2, space="PSUM"))
        ps2 = ctx.enter_context(tc.tile_pool(name="ps2", bufs=4, space="PSUM"))

        w1s = wpool.tile([P, DC, H], F32R)
        nc.sync.dma_start(w1s[:], w1t.rearrange("(dc p) h -> p dc h", p=P))
        w2s = wpool.tile([P, HC, O], F32R)
        nc.sync.dma_start(w2s[:], w2t.rearrange("(hc p) o -> p hc o", p=P))

        NW = NT * NSUB  # tokens per outer tile
        for nt in range(NTILES):
            x_t = xpool.tile([P, DC, NW], F32R)
            nc.sync.dma_start(
                x_t[:],
                xT[:, nt * NW:(nt + 1) * NW].rearrange("(dc p) n -> p dc n", p=P),
            )
            h_t = hpool.tile([P, HC, NW], F32R)
            for hc in range(HC):
                phs = [ps1.tile([P, NT], F32, name=f"ph{s}") for s in range(NSUB)]
                for dc in range(DC):
                    for s in range(NSUB):
                        nc.tensor.matmul(
                            phs[s][:],
                            w1s[:, dc, hc * P:(hc + 1) * P],
                            x_t[:, dc, s * NT:(s + 1) * NT],
                            start=(dc == 0), stop=(dc == DC - 1),
                        )
                for s in range(NSUB):
                    nc.scalar.activation(
                        h_t[:, hc, s * NT:(s + 1) * NT], phs[s][:],
                        mybir.ActivationFunctionType.Relu,
                    )
            for oc in range(OC):
                for s in range(NSUB):
                    po = ps2.tile([P, NT], F32)
                    for hc in range(HC):
                        nc.tensor.matmul(
                            po[:],
                            w2s[:, hc, oc * P:(oc + 1) * P],
                            h_t[:, hc, s * NT:(s + 1) * NT],
                            start=(hc == 0), stop=(hc == HC - 1),
                        )
                    y_t = ypool.tile([P, NT], F32)
                    nc.vector.tensor_copy(y_t[:], po[:])
                    nc.sync.dma_start(
                        yT[oc * P:(oc + 1) * P,
                           nt * NW + s * NT:nt * NW + (s + 1) * NT],
                        y_t[:],
                    )
    nc.compile()
    return nc


def _route(xt, wg):
    """Host-side gate + top-4. Gap between 4th/5th gate values is ~3e-5 for
    this distribution, far above fp32 matmul noise, so fp32 reproduces the
    reference top-k set exactly."""
    gate = xt @ wg  # [N, E] fp32
    top4 = np.argpartition(-gate, TOP_K - 1, axis=1)[:, :TOP_K]  # set, unordered
    return top4


def kernel(x, wg, w1, w2, _want_results=False, _run_kwargs=None):
    B, S, Dx = x.shape
    N = B * S
    xt = np.ascontiguousarray(x.reshape(N, Dx))
    top4 = _route(xt, wg)

    # token lists per expert
    sel = np.zeros((N, E), dtype=bool)
    np.put_along_axis(sel, top4, True, axis=1)
    tokens = [np.nonzero(sel[:, e])[0] for e in range(E)]
    counts = np.array([len(t) for t in tokens])
    CAP = int(-(-counts.max() // (NT * NSUB)) * (NT * NSUB))

    key = CAP
    if key not in _NC_CACHE:
        _NC_CACHE[key] = build_expert_kernel(CAP)
    nc = _NC_CACHE[key]

    in_maps = []
    for e in range(E):
        xe = np.zeros((CAP, Dx), dtype=np.float32)
        xe[:counts[e]] = xt[tokens[e]]
        in_maps.append({
            "xT": np.ascontiguousarray(xe.T),
            "w1t": np.ascontiguousarray(w1[e].T),
            "w2t": np.ascontiguousarray(w2[e].T),
        })

    res = bass_utils.run_bass_kernel_spmd(
        nc, in_maps, core_ids=list(range(E)), **(_run_kwargs or {})
    )

    out = np.zeros((N, O), dtype=np.float32)
    for e in range(E):
        out[tokens[e]] += res.results[e]["yT"].T[:counts[e]]
    out = out.reshape(B, S, O)
    if _want_results:
        return out, res
    return out
